# revision 32
# baseline (speedup 1.0000x reference)
"""Trainium2 Bass kernel: single-head causal attention.

Problem: x[4,2048,1024] f32; q/k/v = x@W* + b* (head dim 128);
out = softmax(causal(q k^T / sqrt(128))) @ v.

Sharding: 8 cores = 4 batches x 2 causal "wedges". Within a batch, the 16
query blocks (128 rows each) are interleaved between the two cores
(h=0 takes odd global blocks, h=1 takes even) so both cores carry an
identical static schedule: slot p attends exactly L_p = 2p+2 local key
blocks. Per-core key order is a host-side permutation of the batch's key
blocks (h=0 identity, h=1 adjacent-pair swap) that puts slot p's own
(diagonal) block at local position 2p+1; the one remaining difference
between wedges (whether local position 2p is a fully-active or a fully
masked block) is carried by a mask *input*, so a single NEFF serves all
8 cores (SPMD).

Per-core pipeline (all on one NeuronCore, Tile-scheduled):
  - k^T / v^T / q^T projections as fp32r matmuls accumulating over the
    8 m-chunks of the 1024 model dim (x^T comes pre-transposed from host,
    weights come pre-chunked so their DMA is contiguous).
  - v^T tiles are PE-transposed into v-natural bf16 tiles augmented with a
    ones column.
  - scores are computed transposed (S^T[k,q]) so that softmax(P^T) tiles
    feed the P@V matmul directly as the stationary operand; softmax uses
    no max-subtraction (scores are O(1) here) and the denominator comes
    for free from the ones column of the augmented V.
"""

import numpy as np

B, T, D, DK = 4, 2048, 1024, 128
NBLK = T // 128      # 16 key blocks per core
NSLOT = 8            # q slots per core (NSLOT*128 = 1024 q rows)
NCHUNK = D // 128    # m-chunks
SCALE = 1.0 / np.sqrt(np.float32(DK))
NEG = -30000.0
JMAJOR = ()          # j-major PV drain hurt: PE stalls on each exp
X_BF16 = True        # pass x / W as bf16: halves input DMA; costs ~input rounding

_built = None


def _build():
    from contextlib import ExitStack

    import concourse.bass as bass
    import concourse.mybir as mybir
    import concourse.tile as tile
    from concourse import bacc
    from concourse.masks import make_identity

    f32 = mybir.dt.float32
    f32r = mybir.dt.float32r
    bf16 = mybir.dt.bfloat16
    Act = mybir.ActivationFunctionType

    nc = bacc.Bacc("TRN2", target_bir_lowering=False, debug=False, num_devices=8)

    xdt = bf16 if X_BF16 else f32r
    xT = nc.dram_tensor("xT", [D, T], xdt, kind="ExternalInput").ap()
    wq = nc.dram_tensor("wq", [128, NCHUNK * DK], xdt, kind="ExternalInput").ap()
    wk = nc.dram_tensor("wk", [128, NCHUNK * DK], xdt, kind="ExternalInput").ap()
    wv = nc.dram_tensor("wv", [128, NCHUNK * DK], xdt, kind="ExternalInput").ap()
    bq = nc.dram_tensor("bq", [DK, 1], f32, kind="ExternalInput").ap()
    bks = nc.dram_tensor("bks", [DK, 1], f32, kind="ExternalInput").ap()  # bk*SCALE
    bv = nc.dram_tensor("bv", [DK, 1], f32, kind="ExternalInput").ap()
    masks = nc.dram_tensor("masks", [128, 256], bf16, kind="ExternalInput").ap()
    o = nc.dram_tensor("o", [NSLOT * 128, DK], f32, kind="ExternalOutput").ap()

    with tile.TileContext(nc) as tc, ExitStack() as ctx:
        const = ctx.enter_context(tc.tile_pool(name="const", bufs=1))
        sbufs = ctx.enter_context(tc.tile_pool(name="sbufs", bufs=1))
        xt_pool = ctx.enter_context(tc.tile_pool(name="xt_pool", bufs=NCHUNK))
        out_pool = ctx.enter_context(tc.tile_pool(name="out_pool", bufs=3))

        # ---- constants (weights come host-pre-chunked: col block c = m-chunk c)
        wk_sb = const.tile([128, NCHUNK * DK], xdt, tag="wk")
        nc.sync.dma_start(out=wk_sb, in_=wk)

        # ---- x^T chunks (kept resident: q projection re-reads them).
        # Order: chunk 0 right after wk so the first k-projection matmul can
        # start as early as possible, then wv, then the remaining chunks.
        xts = []
        for c in range(NCHUNK):
            xt = xt_pool.tile([128, T], xdt, tag="xt", name=f"xt{c}")
            xts.append(xt)

        def load_xt(c):
            nc.sync.dma_start(out=xts[c], in_=xT[128 * c : 128 * (c + 1), :])

        load_xt(0)
        wv_sb = const.tile([128, NCHUNK * DK], xdt, tag="wv")
        nc.sync.dma_start(out=wv_sb, in_=wv)
        for c in range(1, NCHUNK):
            load_xt(c)

        wq_sb = const.tile([128, NCHUNK * DK], xdt, tag="wq")
        nc.sync.dma_start(out=wq_sb, in_=wq)
        bq_sb = const.tile([128, 1], f32, tag="bq")
        nc.sync.dma_start(out=bq_sb, in_=bq)
        bks_sb = const.tile([128, 1], f32, tag="bks")
        nc.sync.dma_start(out=bks_sb, in_=bks)
        bv_sb = const.tile([128, 1], f32, tag="bv")
        nc.sync.dma_start(out=bv_sb, in_=bv)
        mask_sb = const.tile([128, 256], bf16, tag="mask")
        nc.sync.dma_start(out=mask_sb, in_=masks)
        ident = const.tile([128, 128], bf16, tag="ident")
        make_identity(nc, ident)
        # v in natural [k, v] layout, bf16, with a ones column appended
        v_aug = const.tile([128, NBLK, DK + 1], bf16, tag="vaug")
        nc.vector.memset(v_aug[:, :, DK : DK + 1], 1.0)

        # ---- PE warmup: matmuls on a zeroed scratch tile fill the otherwise
        # idle DMA-wait window at kernel start and bring the HAM clock gate to
        # full rate before the real projection matmuls begin.
        WARMUP_MMS = 16
        with tc.tile_pool(name="warmps", bufs=1, space="PSUM") as warmps:
            wsrc = sbufs.tile([128, 512], bf16, tag="wsrc")
            nc.vector.memset(wsrc, 0.0)
            wdst = warmps.tile([128, 512], f32, tag="warm")
            for _ in range(WARMUP_MMS):
                nc.tensor.matmul(
                    wdst, lhsT=wsrc[:, 0:128], rhs=wsrc, start=True, stop=True
                )
            # pull the ~1.3us exp ACT_TABLE_LOAD out of the attention phase
            wexp = sbufs.tile([128, 1], f32, tag="wexp")
            nc.scalar.activation(out=wexp, in_=wsrc[:, 0:1], func=Act.Exp, scale=1.0)

        # ---- projections ----
        kT_sb = sbufs.tile([128, T], bf16, tag="kT")       # (k^T + bk) * SCALE
        qT_sb = sbufs.tile([128, NSLOT * 128], bf16, tag="qT")  # q^T + bq
        vT_sb = sbufs.tile([128, T], bf16, tag="vT")       # v^T + bv

        # kT gets 4 psum banks, qT 2, vT 2 (accumulated in two half-passes) --
        # all three coexist, so no projection matmul ever waits on another
        # projection's psum release.
        kpool = tc.alloc_tile_pool(name="kpool", bufs=1, space="PSUM")
        qpool = tc.alloc_tile_pool(name="qpool", bufs=1, space="PSUM")
        vpool = tc.alloc_tile_pool(name="vpool", bufs=1, space="PSUM")
        if True:
            kT_ps = kpool.tile([128, T], f32, tag="kps")
            qT_ps = qpool.tile([128, NSLOT * 128], f32, tag="qps")
            vTa_ps = vpool.tile([128, T // 2], f32, tag="vps")
            # per chunk: kT x4, vT(first half) x2, qT x2 = 8 matmuls, which
            # matches the x^T chunk DMA arrival rate
            for c in range(NCHUNK):
                for t in range(4):
                    nc.tensor.matmul(
                        kT_ps[:, 512 * t : 512 * (t + 1)],
                        lhsT=wk_sb[:, 128 * c : 128 * (c + 1)],
                        rhs=xts[c][:, 512 * t : 512 * (t + 1)],
                        start=(c == 0),
                        stop=(c == NCHUNK - 1),
                    )
                for t in range(2):
                    nc.tensor.matmul(
                        vTa_ps[:, 512 * t : 512 * (t + 1)],
                        lhsT=wv_sb[:, 128 * c : 128 * (c + 1)],
                        rhs=xts[c][:, 512 * t : 512 * (t + 1)],
                        start=(c == 0),
                        stop=(c == NCHUNK - 1),
                    )
                x4 = xts[c].rearrange("p (b two x) -> p b two x", two=2, x=128)
                for t in range(2):
                    nc.tensor.matmul(
                        qT_ps[:, 512 * t : 512 * (t + 1)],
                        lhsT=wq_sb[:, 128 * c : 128 * (c + 1)],
                        rhs=x4[:, 4 * t : 4 * t + 4, 1, :],
                        start=(c == 0),
                        stop=(c == NCHUNK - 1),
                    )
            # copies: kT+qT on ACT (score deps), vT halves on DVE
            for t in range(2):
                sl = slice(1024 * t, 1024 * (t + 1))
                nc.scalar.activation(
                    out=kT_sb[:, sl], in_=kT_ps[:, sl], func=Act.Identity,
                    bias=bks_sb, scale=SCALE,
                )
            for t in range(2):
                sl = slice(512 * t, 512 * (t + 1))
                nc.scalar.activation(
                    out=qT_sb[:, sl], in_=qT_ps[:, sl], func=Act.Identity,
                    bias=bq_sb, scale=1.0,
                )
            for t in range(2):
                sl = slice(512 * t, 512 * (t + 1))
                nc.vector.tensor_scalar_add(vT_sb[:, sl], vTa_ps[:, sl], bv_sb)

            # vT second half accumulates while the kT/qT copies drain
            vTb_ps = vpool.tile([128, T // 2], f32, tag="vps")
            for c in range(NCHUNK):
                for t in range(2):
                    nc.tensor.matmul(
                        vTb_ps[:, 512 * t : 512 * (t + 1)],
                        lhsT=wv_sb[:, 128 * c : 128 * (c + 1)],
                        rhs=xts[c][:, 1024 + 512 * t : 1024 + 512 * (t + 1)],
                        start=(c == 0),
                        stop=(c == NCHUNK - 1),
                    )
            for t in range(2):
                sl = slice(512 * t, 512 * (t + 1))
                nc.vector.tensor_scalar_add(
                    vT_sb[:, 1024 + 512 * t : 1024 + 512 * (t + 1)],
                    vTb_ps[:, sl], bv_sb,
                )

        # ---- attention ----
        vpool.release()
        qpool.release()
        kpool.release()
        spool = ctx.enter_context(tc.tile_pool(name="spool", bufs=3, space="PSUM"))
        # one shared 5-slot pool for transpose scratch AND output accumulators:
        # transposes need slots early in the attention phase, o_ps late, so a
        # shared pool gives each phase more slack than a static 1/4 split
        opool = ctx.enter_context(tc.tile_pool(name="opool", bufs=5, space="PSUM"))
        pt_pool = ctx.enter_context(tc.tile_pool(name="pt_pool", bufs=NBLK))

        # v^T -> v natural (bf16) via PE transpose; emitted lazily inside the
        # attention loop so the PE never stalls in a transpose block waiting
        # for the vT copies (transpose for key block j lands just before its
        # S^T matmul; burst p only needs transposes <= 2p+1, which are done)
        def emit_transpose(j):
            vt_ps = opool.tile([128, DK + 1], bf16, tag="o", name=f"vt_ps{j}")
            vt_ps = vt_ps[:, 0:128]
            nc.tensor.transpose(vt_ps, vT_sb[:, 128 * j : 128 * (j + 1)], ident)
            nc.vector.tensor_copy(v_aug[:, j, 0:DK], vt_ps)

        def chunk_sizes(n):
            # pieces <=512, all >=256 when possible (fp32r full-rate needs >=256)
            out = []
            while n > 768:
                out.append(512)
                n -= 512
            if n > 512:
                out.append(n - 256)
                n = 256
            out.append(n)
            return out

        pts = [None] * NBLK

        def pv_mm(o_ps, p, jj):
            nc.tensor.matmul(
                o_ps,
                lhsT=pts[jj][:, 128 * (p - jj // 2) : 128 * (p - jj // 2) + 128],
                rhs=v_aug[:, jj, :],
                start=(jj == 0),
                stop=(jj == 2 * p + 1),
            )

        def finish_slot(o_ps, p):
            rcp = out_pool.tile([128, 1], f32, tag="rcp")
            nc.vector.reciprocal(rcp, o_ps[:, DK : DK + 1])
            ob = out_pool.tile([128, DK], f32, tag="ob")
            nc.vector.tensor_scalar_mul(ob, o_ps[:, 0:DK], rcp)
            nc.sync.dma_start(out=o[128 * p : 128 * (p + 1), :], in_=ob)

        # process key positions 14,15 early so the final P@V bursts never
        # wait on their exp at the very end of the kernel
        ORDER = [0, 1, 2, 3, 4, 5, 6, 7, 8, 9, 14, 15, 10, 11, 12, 13]
        done = set()
        burst_done = set()
        for j in ORDER:
            sj = j // 2           # first active slot for this key position
            q0 = 128 * sj
            qn = NSLOT * 128 - q0
            pt = pt_pool.tile([128, qn], bf16, tag="pt", name=f"pt{j}")
            pts[j] = pt
            off = 0
            for sz in chunk_sizes(qn):
                s_ps = spool.tile([128, 512], f32, tag="st")
                nc.tensor.matmul(
                    s_ps[:, :sz],
                    lhsT=kT_sb[:, 128 * j : 128 * (j + 1)],
                    rhs=qT_sb[:, q0 + off : q0 + off + sz],
                    start=True,
                    stop=True,
                )
                nc.scalar.activation(
                    out=pt[:, off : off + sz], in_=s_ps[:, :sz], func=Act.Exp,
                    scale=1.0,
                )
                if off == 0:
                    # mask the frontier slot multiplicatively (exp(s+m) =
                    # exp(s)*m01): even j -> maskA (wedge-dependent), odd j ->
                    # maskB (causal triangle); bf16 SBUF op, off the psum path
                    sel = j % 2
                    nc.vector.tensor_mul(
                        pt[:, 0:128],
                        pt[:, 0:128],
                        mask_sb[:, 128 * sel : 128 * (sel + 1)],
                    )
                off += sz

            emit_transpose(j)
            done.add(j)
            for p in range(NSLOT):
                if p not in burst_done and all(
                    jj in done for jj in range(2 * p + 2)
                ):
                    burst_done.add(p)
                    o_ps = opool.tile([128, DK + 1], f32, tag="o", name=f"o_ps{p}")
                    for jj in range(2 * p + 2):
                        pv_mm(o_ps, p, jj)
                    finish_slot(o_ps, p)

    nc.compile()
    return nc


def get_built():
    global _built
    if _built is None:
        _built = _build()
    return _built


def _pos2glob(h):
    if h == 0:
        return list(range(NBLK))
    return [j + 1 if j % 2 == 0 else j - 1 for j in range(NBLK)]


def _xdt():
    if X_BF16:
        import ml_dtypes
        return ml_dtypes.bfloat16
    return np.float32


def _pack_w(W):
    """[D, DK] -> [128, NCHUNK*DK] with column block c holding rows 128c..128c+127."""
    return np.ascontiguousarray(
        np.asarray(W, np.float32).reshape(NCHUNK, 128, DK).transpose(1, 0, 2)
        .reshape(128, NCHUNK * DK).astype(_xdt())
    )


def make_in_map(x_b, Wq, bq, Wk, bk, Wv, bv, h, xT_pre=None):
    """Build one core's input dict. x_b: [T, D] fp32 for this core's batch.
    xT_pre: optional precomputed x_b.T already in the kernel dtype (shared by
    both wedge cores of a batch; h=0 uses it as-is, h=1 column-permutes)."""
    if xT_pre is None:
        xT_pre = np.ascontiguousarray(x_b.T.astype(_xdt()))
    if h == 0:
        xT_loc = xT_pre  # identity key order
    else:
        p2g = _pos2glob(h)
        cols = np.concatenate([np.arange(128 * g, 128 * (g + 1)) for g in p2g])
        xT_loc = np.ascontiguousarray(xT_pre[:, cols])
    import ml_dtypes
    bf = ml_dtypes.bfloat16
    maskA = (np.ones if h == 0 else np.zeros)((128, 128), bf)
    kk = np.arange(128)
    maskB = np.where(kk[:, None] <= kk[None, :], 1.0, 0.0).astype(bf)
    return {
        "xT": xT_loc,
        "wq": _pack_w(Wq),
        "wk": _pack_w(Wk),
        "wv": _pack_w(Wv),
        "bq": np.ascontiguousarray(bq.reshape(DK, 1), np.float32),
        "bks": np.ascontiguousarray((bk * SCALE).reshape(DK, 1), np.float32),
        "bv": np.ascontiguousarray(bv.reshape(DK, 1), np.float32),
        "masks": np.ascontiguousarray(np.concatenate([maskA, maskB], axis=1)),
    }


def gather_out(results):
    """results: list of 8 dicts with 'o' [1024, 128] -> full [B, T, DK]."""
    out = np.zeros((B, T, DK), np.float32)
    for core in range(8):
        b, h = core // 2, core % 2
        ob = results[core]["o"]
        for p in range(NSLOT):
            g = 2 * p + 1 - h
            out[b, 128 * g : 128 * (g + 1), :] = ob[128 * p : 128 * (p + 1), :]
    return out


def kernel(x, Wq, bq, Wk, bk, Wv, bv):
    from concourse.bass_utils import run_bass_kernel_spmd

    x = np.asarray(x, np.float32)
    args = [np.asarray(a, np.float32) for a in (Wq, bq, Wk, bk, Wv, bv)]
    nc = get_built()
    # one transpose+cast per batch, shared by its two wedge cores
    xT_pres = [np.ascontiguousarray(x[b].T.astype(_xdt())) for b in range(B)]
    in_maps = [
        make_in_map(x[core // 2], args[0], args[1], args[2], args[3], args[4],
                    args[5], core % 2, xT_pre=xT_pres[core // 2])
        for core in range(8)
    ]
    res = run_bass_kernel_spmd(nc, in_maps, core_ids=list(range(8)))
    return gather_out(res.results)


if __name__ == "__main__":
    rng = np.random.default_rng(0)
    x = rng.standard_normal((B, T, D), dtype=np.float32)
    Wq = rng.standard_normal((D, DK), dtype=np.float32) * 0.03
    out = kernel(x, Wq, np.zeros(DK, np.float32), Wq, np.zeros(DK, np.float32),
                 Wq, np.zeros(DK, np.float32))
    print(out.shape)
